# revision 10
# baseline (speedup 1.0000x reference)
"""APPNP (K-step propagation over an MLP) on 8 TRN2 NeuronCores.

Strategy:
  - Nodes are partitioned contiguously across the 8 cores (12500 per core,
    padded to 12544 = 98*128 device-side slots). Within each core, nodes are
    RELABELED in descending in-degree order so that fixed-degree padding per
    128-node block is cheap.
  - The dense MLP (lin1/lin2) is data-parallel over nodes.
  - Propagation uses A_hat h = dinv * (A g + g), g = dinv * h, so there is no
    per-edge scaling. Each step: AllGather of the g shard into a full table,
    then a fixed-degree pull: for block t and in-edge slot j, one indirect DMA
    fetches g[src] for the 128 nodes of the block (one row per partition).
    Slot rows are reduced on the vector engine into per-octet partial sums,
    then per-block sums; a small per-node epilogue forms the next g.
  - log_softmax at the end on the local shard; host un-permutes rows.
"""

import sys

sys.path.insert(0, "/opt/trn_rl_repo")

import numpy as np

NCORES = 8
ALPHA = 0.1
F_IN = 512
HID = 64
C = 40
CP = 64  # padded class dim; table rows are CP floats = 256B
OCT = 16  # slots per octet
OCT_PER_BODY = 4  # octets per For_i iteration
OSUM_DT_F32 = False  # debug switch
PER_STEP_TILES = False  # debug switch


class Cfg:
    def __init__(self, N, LV, L, K):
        assert N == NCORES * LV
        assert L % 128 == 0 and LV < L
        self.N = N
        self.LV = LV
        self.L = L
        self.K = K
        self.T = L // 128
        self.NPAD = NCORES * L


FULL = Cfg(N=100000, LV=12500, L=12544, K=10)


def preprocess(cfg, x, edge_index, w1, b1, w2, b2):
    N, LV, L, T = cfg.N, cfg.LV, cfg.L, cfg.T
    src = np.asarray(edge_index[0], dtype=np.int64)
    dst = np.asarray(edge_index[1], dtype=np.int64)

    deg_in = np.bincount(dst, minlength=N)
    deg = (deg_in + 1).astype(np.float32)
    dinv = (1.0 / np.sqrt(deg)).astype(np.float32)

    # per-core relabeling: slot s of core r holds node ord_g[r][s] (global id)
    ord_g = np.empty((NCORES, LV), dtype=np.int64)
    slot_of = np.empty(N, dtype=np.int64)
    for r in range(NCORES):
        ids = np.arange(r * LV, (r + 1) * LV, dtype=np.int64)
        o = ids[np.argsort(-deg_in[ids], kind="stable")]
        ord_g[r] = o
        slot_of[o] = np.arange(LV, dtype=np.int64)

    # table row of a global node in the relabeled padded table
    trow = (np.arange(N) // LV) * L + slot_of

    # per-block slot counts (cross-core max, octet-rounded)
    Dmax_blocks = np.zeros(T, dtype=np.int64)
    for r in range(NCORES):
        d_slots = np.zeros(L, dtype=np.int64)
        d_slots[:LV] = deg_in[ord_g[r]]
        bm = d_slots.reshape(T, 128).max(axis=1)
        Dmax_blocks = np.maximum(Dmax_blocks, bm)
    noct_arr = np.maximum(1, np.ceil(Dmax_blocks / OCT).astype(np.int64))
    total = int(noct_arr.sum())
    noct_arr[T - 1] += (-total) % OCT_PER_BODY
    noct = [int(v) for v in noct_arr]
    ooff = np.concatenate([[0], np.cumsum(noct_arr)]).astype(np.int64)
    NOCT = int(ooff[-1])
    SLOTS = NOCT * OCT

    # edges sorted by dst for per-node in-edge lists
    order_e = np.argsort(dst, kind="stable")
    dst_s = dst[order_e]
    src_s = src[order_e]
    starts = np.zeros(N + 1, dtype=np.int64)
    starts[1:] = np.cumsum(deg_in)

    w2p = np.zeros((HID, CP), dtype=np.float32)
    w2p[:, :C] = np.asarray(w2, dtype=np.float32)
    b2p = np.zeros((CP,), dtype=np.float32)
    b2p[:C] = np.asarray(b2, dtype=np.float32)
    w1s = np.asarray(w1, dtype=np.float32).reshape(4, 128, HID).transpose(1, 0, 2).copy()
    b1r = np.tile(np.asarray(b1, dtype=np.float32), (128, 1))
    b2r = np.tile(b2p, (128, 1))
    ident = np.eye(128, dtype=np.float32)

    xf = np.asarray(x, dtype=np.float32)
    in_maps = []
    for r in range(NCORES):
        ZR = r * L + LV  # guaranteed-zero table row of our own rank
        Wmax = max(noct) * OCT
        pad = np.full((L, Wmax), ZR, dtype=np.int64)
        ids = ord_g[r]
        lo, hi = np.searchsorted(dst_s, [r * LV, (r + 1) * LV])
        dsts_r = dst_s[lo:hi]
        srcs_r = src_s[lo:hi]
        pos_r = np.arange(lo, hi) - starts[dsts_r]
        rows_r = slot_of[dsts_r]
        pad[rows_r, pos_r] = trow[srcs_r]

        idx32 = np.full((128, SLOTS), ZR, dtype=np.int32)
        for t in range(T):
            w = noct[t] * OCT
            idx32[:, int(ooff[t]) * OCT : int(ooff[t]) * OCT + w] = pad[
                t * 128 : (t + 1) * 128, :w
            ]

        xs = np.zeros((L, F_IN), dtype=np.float32)
        xs[:LV] = xf[ids]
        xt = np.ascontiguousarray(xs.T)

        dl = np.zeros((L,), dtype=np.float32)
        dl[:LV] = dinv[ids]
        dpt = np.ascontiguousarray(dl.reshape(T, 128).T)

        in_maps.append(
            {
                "xt": xt,
                "w1s": w1s,
                "b1r": b1r,
                "w2p": w2p,
                "b2r": b2r,
                "ident": ident,
                "dinv": dpt,
                "d1a": ((1.0 - ALPHA) * dpt).astype(np.float32),
                "d2a": ((1.0 - ALPHA) * dpt * dpt).astype(np.float32),
                "gidx": idx32,
            }
        )
    meta = {
        "noct": noct,
        "ooff": [int(v) for v in ooff],
        "NOCT": NOCT,
        "SLOTS": SLOTS,
        "ord_g": ord_g,
    }
    return in_maps, meta


def build(cfg, meta):
    import concourse.bacc as bacc
    import concourse.bass as bass
    import concourse.mybir as mybir
    import concourse.tile as tile

    fp32 = mybir.dt.float32
    bf16 = mybir.dt.bfloat16
    i32 = mybir.dt.int32
    AF = mybir.ActivationFunctionType
    ALU = mybir.AluOpType

    L, T, K, NPAD = cfg.L, cfg.T, cfg.K, cfg.NPAD
    noct, ooff, NOCT, SLOTS = meta["noct"], meta["ooff"], meta["NOCT"], meta["SLOTS"]

    nc = bacc.Bacc("TRN2", target_bir_lowering=False, debug=False, num_devices=NCORES)

    xt_e = nc.declare_dram_parameter("xt", [F_IN, L], fp32, isOutput=False)
    w1s_e = nc.declare_dram_parameter("w1s", [128, 4, HID], fp32, isOutput=False)
    b1r_e = nc.declare_dram_parameter("b1r", [128, HID], fp32, isOutput=False)
    w2p_e = nc.declare_dram_parameter("w2p", [HID, CP], fp32, isOutput=False)
    b2r_e = nc.declare_dram_parameter("b2r", [128, CP], fp32, isOutput=False)
    ident_e = nc.declare_dram_parameter("ident", [128, 128], fp32, isOutput=False)
    dinv_e = nc.declare_dram_parameter("dinv", [128, T], fp32, isOutput=False)
    d1a_e = nc.declare_dram_parameter("d1a", [128, T], fp32, isOutput=False)
    d2a_e = nc.declare_dram_parameter("d2a", [128, T], fp32, isOutput=False)
    gidx_e = nc.declare_dram_parameter("gidx", [128, SLOTS], i32, isOutput=False)
    out_e = nc.declare_dram_parameter("out", [L, C], fp32, isOutput=True)

    with tile.TileContext(nc) as tc:
        with (
            tc.tile_pool(name="res", bufs=1) as res,
            tc.tile_pool(name="dram", bufs=1, space="DRAM") as dram,
            tc.tile_pool(name="mlp", bufs=3) as mlp,
            tc.tile_pool(name="mpsum", bufs=2, space="PSUM") as mpsum,
            tc.tile_pool(name="lp", bufs=1) as lp,
        ):
            g_cur = res.tile([128, T, CP], fp32)
            a_g0 = res.tile([128, T, CP], bf16)
            a_h0 = res.tile([128, T, CP], bf16)
            osum_dt = fp32 if OSUM_DT_F32 else bf16
            if not PER_STEP_TILES:
                osum = res.tile([128, NOCT, CP], osum_dt)
            it = res.tile([128, SLOTS], i32)
            w1_sb = res.tile([128, 4, HID], fp32)
            b1_sb = res.tile([128, HID], fp32)
            w2_sb = res.tile([HID, CP], fp32)
            b2_sb = res.tile([128, CP], fp32)
            id_sb = res.tile([128, 128], fp32)
            dinv_sb = res.tile([128, T], fp32)
            d1a_sb = res.tile([128, T], fp32)
            d2a_sb = res.tile([128, T], fp32)

            nc.sync.dma_start(out=it[:], in_=gidx_e[:, :])
            nc.sync.dma_start(out=w1_sb[:], in_=w1s_e[:, :, :])
            nc.sync.dma_start(out=b1_sb[:], in_=b1r_e[:, :])
            nc.sync.dma_start(out=w2_sb[:], in_=w2p_e[:, :])
            nc.sync.dma_start(out=b2_sb[:], in_=b2r_e[:, :])
            nc.sync.dma_start(out=id_sb[:], in_=ident_e[:, :])
            nc.sync.dma_start(out=dinv_sb[:], in_=dinv_e[:, :])
            nc.sync.dma_start(out=d1a_sb[:], in_=d1a_e[:, :])
            nc.sync.dma_start(out=d2a_sb[:], in_=d2a_e[:, :])

            xt_r = xt_e.ap().rearrange("(kb p) n -> p kb n", p=128)

            # ---- MLP
            for t in range(T):
                xk = mlp.tile([128, 4, 128], fp32, tag="xk")
                nc.sync.dma_start(out=xk[:], in_=xt_r[:, :, t * 128 : (t + 1) * 128])
                ps1 = mpsum.tile([128, HID], fp32, tag="ps1")
                for k in range(4):
                    nc.tensor.matmul(
                        ps1[:], xk[:, k, :], w1_sb[:, k, :],
                        start=(k == 0), stop=(k == 3),
                    )
                h1 = mlp.tile([128, HID], fp32, tag="h1")
                nc.vector.tensor_tensor(h1[:], ps1[:], b1_sb[:], op=ALU.add)
                nc.scalar.activation(h1[:], h1[:], AF.Relu)
                pst = mpsum.tile([128, 128], fp32, tag="pst")
                nc.tensor.transpose(pst[:HID, :], h1[:], id_sb[:])
                h1t = mlp.tile([HID, 128], fp32, tag="h1t")
                nc.vector.tensor_copy(h1t[:], pst[:HID, :])
                ps2 = mpsum.tile([128, CP], fp32, tag="ps2")
                nc.tensor.matmul(ps2[:], h1t[:], w2_sb[:], start=True, stop=True)
                h0t = mlp.tile([128, CP], fp32, tag="h0t")
                nc.vector.tensor_tensor(h0t[:], ps2[:], b2_sb[:], op=ALU.add)
                with nc.allow_low_precision(reason="alpha anchors stored bf16"):
                    nc.vector.tensor_scalar_mul(a_h0[:, t, :], h0t[:], ALPHA)
                nc.vector.tensor_scalar(
                    g_cur[:, t, :], h0t[:], dinv_sb[:, t : t + 1], None, op0=ALU.mult
                )
                with nc.allow_low_precision(reason="alpha anchors stored bf16"):
                    nc.vector.tensor_scalar_mul(a_g0[:, t, :], g_cur[:, t, :], ALPHA)

            rg = [list(range(NCORES))]
            trips = NOCT // OCT_PER_BODY
            for step in range(1, K + 1):
                ag_in = dram.tile([L, CP], fp32, name=f"agi{step}", tag=f"agi{step}")
                ag_out = dram.tile(
                    [NPAD, CP], fp32, addr_space="Shared",
                    name=f"ago{step}", tag=f"ago{step}",
                )
                ag_in_r = ag_in[:].rearrange("(t p) c -> p t c", p=128)
                nc.sync.dma_start(out=ag_in_r, in_=g_cur[:])
                nc.gpsimd.collective_compute(
                    "AllGather",
                    mybir.AluOpType.bypass,
                    replica_groups=rg,
                    ins=[ag_in.opt()],
                    outs=[ag_out.opt()],
                )

                if PER_STEP_TILES:
                    osum = res.tile([128, NOCT, CP], osum_dt,
                                    name=f"osum{step}", tag=f"osum{step}")
                NBUF = 4
                stages = [
                    lp.tile([128, OCT], i32, name=f"st{step}_{k}",
                            tag=f"st{k}" if not PER_STEP_TILES else f"st{step}_{k}")
                    for k in range(NBUF)
                ]
                gts = [
                    lp.tile([128, OCT, CP], fp32, name=f"gt{step}_{k}",
                            tag=f"gt{k}" if not PER_STEP_TILES else f"gt{step}_{k}")
                    for k in range(NBUF)
                ]
                with tc.For_i(0, trips, 1) as i:
                    for k in range(OCT_PER_BODY):
                        stage = stages[k % NBUF]
                        gt = gts[k % NBUF]
                        nc.vector.tensor_copy(
                            stage[:],
                            it[:, bass.ds(i * (OCT_PER_BODY * OCT) + k * OCT, OCT)],
                        )
                        for l in range(OCT):
                            nc.gpsimd.indirect_dma_start(
                                out=gt[:, l, :],
                                out_offset=None,
                                in_=ag_out[:],
                                in_offset=bass.IndirectOffsetOnAxis(
                                    ap=stage[:, l : l + 1], axis=0
                                ),
                            )
                        gre = bass.AP(
                            gt[:].tensor, gt[:].offset,
                            [gt[:].ap[0], [1, CP], [CP, OCT]],
                        )
                        with nc.allow_low_precision(reason="octet partials bf16"):
                            nc.vector.tensor_reduce(
                                osum[:, bass.ds(i * OCT_PER_BODY + k, 1), :],
                                gre,
                                axis=mybir.AxisListType.X,
                                op=ALU.add,
                            )

                # per-block reduce + epilogue
                last = step == K
                dsc = d1a_sb if last else d2a_sb
                anchor = a_h0 if last else a_g0
                for t in range(T):
                    n = noct[t]
                    agg = mlp.tile([128, CP], fp32, tag="agg")
                    if n == 1:
                        nc.vector.tensor_copy(agg[:], osum[:, ooff[t], :])
                    else:
                        ore = bass.AP(
                            osum[:].tensor,
                            osum[:].offset + ooff[t] * CP,
                            [osum[:].ap[0], [1, CP], [CP, n]],
                        )
                        nc.vector.tensor_reduce(
                            agg[:], ore, axis=mybir.AxisListType.X, op=ALU.add
                        )
                    nc.vector.tensor_tensor(agg[:], agg[:], g_cur[:, t, :], op=ALU.add)
                    nc.vector.scalar_tensor_tensor(
                        g_cur[:, t, :],
                        agg[:],
                        dsc[:, t : t + 1],
                        anchor[:, t, :],
                        op0=ALU.mult,
                        op1=ALU.add,
                    )

            # ---- log_softmax over first C cols of g_cur (== h_K)
            red = res.tile([128, T, 2], fp32)
            ex = res.tile([128, T, C], fp32)
            nc.vector.tensor_reduce(
                red[:, :, 0:1], g_cur[:, :, 0:C], axis=mybir.AxisListType.X, op=ALU.max
            )
            for t in range(T):
                nc.vector.tensor_scalar(
                    ex[:, t, :], g_cur[:, t, 0:C], red[:, t, 0:1], None,
                    op0=ALU.subtract,
                )
            nc.scalar.activation(ex[:], ex[:], AF.Exp)
            nc.vector.tensor_reduce(
                red[:, :, 1:2], ex[:], axis=mybir.AxisListType.X, op=ALU.add
            )
            nc.scalar.activation(red[:, :, 1:2], red[:, :, 1:2], AF.Ln)
            outt = res.tile([128, T, C], fp32)
            for t in range(T):
                nc.vector.tensor_scalar(
                    outt[:, t, :], g_cur[:, t, 0:C], red[:, t, 0:1], red[:, t, 1:2],
                    op0=ALU.subtract, op1=ALU.subtract,
                )
            out_r = out_e.ap().rearrange("(t p) c -> p t c", p=128)
            nc.sync.dma_start(out=out_r, in_=outt[:])

    nc.finalize()
    return nc


def _run(cfg, x, edge_index, w1, b1, w2, b2, trace=False):
    from concourse import bass_utils

    in_maps, meta = preprocess(cfg, x, edge_index, w1, b1, w2, b2)
    nc = build(cfg, meta)
    res = bass_utils.run_bass_kernel_spmd(
        nc, in_maps, core_ids=list(range(NCORES)), trace=trace
    )
    out = np.empty((cfg.N, C), dtype=np.float32)
    for r in range(NCORES):
        out[meta["ord_g"][r]] = res.results[r]["out"][: cfg.LV]
    return out, res


def kernel(x, edge_index, w1, b1, w2, b2):
    out, _ = _run(
        FULL,
        np.asarray(x),
        np.asarray(edge_index),
        np.asarray(w1),
        np.asarray(b1),
        np.asarray(w2),
        np.asarray(b2),
    )
    return out
